# revision 2
# baseline (speedup 1.0000x reference)
"""GatingMixedDecoder Trainium2 kernel.

Accepts FULL unsharded inputs, returns FULL [4096, 512] f32 output.
Data-parallel over batch: 8 NeuronCores x 512 rows, expert weights replicated.
Per-core Bass program: soft-MoE decoder forward with LN folded into matmul
corrections and expert mixing folded into per-expert scaling of x^T tiles, so
each 128x512 output tile is a single PSUM accumulation over (expert, feature)
chunks plus two K=8 correction matmuls. All matmul operands bf16, fp32 PSUM.
"""
import numpy as np

N_CORES = 8
B_FULL = 4096
B_LOCAL = B_FULL // N_CORES   # 512
NBC = B_LOCAL // 128

_nc_cache = {}


def _get_nc():
    if "nc" not in _nc_cache:
        import kernel_build as kb
        _nc_cache["nc"] = kb.build(nbc=NBC)
    return _nc_cache["nc"]


def kernel(**inputs: np.ndarray) -> np.ndarray:
    import kernel_build as kb
    from concourse.bass_utils import run_bass_kernel_spmd

    nc = _get_nc()
    w = kb.prep_weights(inputs)
    z = np.asarray(inputs["z"], np.float32)
    c = np.asarray(inputs["c"], np.float32)

    in_maps = []
    for i in range(N_CORES):
        sl = slice(i * B_LOCAL, (i + 1) * B_LOCAL)
        in_maps.append({**w, **kb.prep_shard(z[sl], c[sl])})

    res = run_bass_kernel_spmd(nc, in_maps, list(range(N_CORES)))
    out = np.concatenate([res.results[i]["out"] for i in range(N_CORES)], axis=0)
    return np.ascontiguousarray(out.astype(np.float32))
